# revision 28
# baseline (speedup 1.0000x reference)
"""AnalyticGaussianVelocity (soft-kNN flow velocity) on 8 trn2 NeuronCores.

Math (reference):
    a = t, b = 1-t
    logit[b,n] = -1/(2 b^2) * ||x_b - a * d_n||^2
    prob = softmax(logit, axis=n) * (1 + a/b)
    v = (-1/b) x + prob @ dataset

Dropping per-row constants, softmax(logit) == softmax(u * P) with
    u = a/b^2  (>0),  P[b,n] = x_b . d_n - (a/2) ||d_n||^2

Kernel strategy (dataset sharded over N across 8 cores, flash-style
online softmax per core, AllReduce merge):
  host prep: dataset transposed (dT fp32 [D,n]), bf16 copy (natb [n,D]),
       norms dn = ||d_n||^2 with 3-way bf16 splits packed as dn6 [6,n],
       w = -(a/2) 3-way bf16 split packed as w6 [6,B], xT fp32 [D,B].
  MM1: P = xT.T @ dT as a SINGLE float32r pass (hw-validated: f32r keeps
       11 explicit mantissa bits, 1 cyc/row; end-to-end err 1.6e-3)
       + a K=6 bf16 matmul folding in the -(a/2)||d||^2 term
       (w1*dn1+w1*dn2+w1*dn3+w2*dn1+w2*dn2+w3*dn1 ~ full fp32 product).
       f32r operands are rounded on device (compiler requires a rounding
       producer): dT chunks on Pool, xT at setup.
  softmax: DVE row-max -> ACT exp(scale=u, bias=-u*m) with free row-sum
       (accum_out), prob emitted in bf16.
  MM2: pA = probT @ natb (bf16, PSUM); then one DVE scalar_tensor_tensor
       does acc = alpha*acc + pA (replaces the diag-rescale matmul and
       the acc copy). probT via PE transposes + ACT copy back to SBUF.
  schedule: software-pipelined per b-tile block: iteration j emits
       MM1(j), then the softmax chain of block j-1, with MM2 lagging
       ~4 blocks so PE never waits on the DVE/ACT chain or probT copy.
  PIPE_V3 (default): n-tiles processed in PAIRS — one [128,1024] pL
       spanning 2 PSUM banks per (b-tile, pair). Halves the count of
       reduce/exp/smalls/acc instructions (the HW loses ~2x to
       per-instruction sync overheads that the cost model doesn't see),
       and each MM1 weight load covers two streams (A/B back-to-back
       under the same lhsT). PSUM: pL 2x2 + pA 2 + pP(bf16) 2 = 8 banks.
       HW-measured ~590us vs ~810us for the per-tile v1 pipeline.
  merge: AllReduce-max of m, rescale by exp(u(m_loc-m_glob)),
         AllReduce-add of [acc | l], then v = dcoef*acc/l + vcoef*x.
  timing support: build(n_tiles, repeat=R) wraps the main loop in a
       hardware For_i loop (merge stays outside: collectives inside
       For_i crash the device) so real HW time is measurable as a slope
       over R despite the ~100ms axon RPC turnaround.
"""

import sys

sys.path.insert(0, "/opt/trn_rl_repo")

import numpy as np
import ml_dtypes

import concourse.bass as bass
import concourse.mybir as mybir
import concourse.tile as tile
from concourse import bacc
from concourse.bass_utils import run_bass_kernel_spmd
from concourse.masks import make_identity

B, D = 1024, 512
NCORES = 8
NTILE = 512  # dataset rows per n-tile
NBT = B // 128  # 8 b-tiles

F32 = mybir.dt.float32
F32R = mybir.dt.float32r
F16 = mybir.dt.float16
BF16 = mybir.dt.bfloat16

AF = mybir.ActivationFunctionType
OP = mybir.AluOpType
AX = mybir.AxisListType

SIM_1CORE = False  # build single-core, no collectives (for TimelineSim)
LINEARIZE = False
ROUND_ENGINE = "pool"  # engine for fp32 -> f32r rounding copies
PROBT_ENGINE = "act"  # engine for probT psum -> sbuf copy
ACC_ENGINE = "act"  # engine for acc psum -> sbuf copy
USE_XBAR = False  # DMA-transpose for probT (else PE transposes + copy)
STRIP = "full"  # timing bisection: "mm1" | "mm1bf" | "stats" | "probt" | "full"
PIPE_V2 = False  # stage-split pipeline (no fresh cross-engine deps per slot)
PIPE_V3 = True  # paired n-tiles: one [128,1024] pL per (b-tile, tile-pair)
V3_STATS_LAG = 1  # pairs of lag for stats chain
V3_MM2_LAG = 2  # pairs of lag for MM2+acc
V3_SPLITB = False  # emit alpha+l_run in the MM2 slot (2 pairs later)
FP16_DT = True  # ship dT/xr as fp16: half dT DMA, no Pool rounding copy
V3_LATEACC = True  # run the acc STT one slot after its MM2 (no DVE wait on PE)
V3_COPYSPLIT = False  # probT copy: first half on ACT, second half on DVE
BUFS_DT = 2
BUFS_NAT = 3
BUFS_SF = 7
BUFS_TINY = 8
BUFS_PSL = 3
STATS_LAG = 1
TRANS_LAG = 2
MM2_LAG = 3
FUSE_TRANS = True
SMALL_ON_POOL = False  # gpsimd tensor ops fail walrus lowering; keep DVE
BUFS_PSA = 2
BUFS_PSP = 2


def _copy(nc, eng, dst, src):
    if eng == "pool":
        nc.gpsimd.tensor_copy(dst, src)
    elif eng == "dve":
        nc.vector.tensor_copy(dst, src)
    else:
        nc.scalar.copy(dst, src)


def build(n_tiles, repeat=1, merge_repeat=1):
    n_sh = n_tiles * NTILE
    ndev = 1 if SIM_1CORE else NCORES
    nc = bacc.Bacc("TRN2", target_bir_lowering=False, debug=False, num_devices=ndev)

    dT_p = nc.declare_dram_parameter("dT", [D, n_sh], F16 if FP16_DT else F32, isOutput=False)
    natb_p = nc.declare_dram_parameter("natb", [n_sh, D], BF16, isOutput=False)
    dn6_p = nc.declare_dram_parameter("dn6", [6, n_sh], BF16, isOutput=False)
    w6_p = nc.declare_dram_parameter("w6", [6, B], BF16, isOutput=False)
    xT_p = nc.declare_dram_parameter("xT", [D, B], F32, isOutput=False)
    xrow_p = nc.declare_dram_parameter("xrow", [B, D], F32, isOutput=False)
    # per-b coefficient vectors, column layout [128, 8]: col i holds b = i*128+p
    ucol_p = nc.declare_dram_parameter("ucol", [128, NBT], F32, isOutput=False)
    nucol_p = nc.declare_dram_parameter("nucol", [128, NBT], F32, isOutput=False)
    dcol_p = nc.declare_dram_parameter("dcol", [128, NBT], F32, isOutput=False)
    vcol_p = nc.declare_dram_parameter("vcol", [128, NBT], F32, isOutput=False)
    out = nc.declare_dram_parameter("out", [B, D], F32, isOutput=True)

    dT_t = dT_p.ap().rearrange("(k p) (t n) -> t p k n", p=128, n=NTILE)
    natb_t = natb_p.ap().rearrange("(t j p) d -> t p j d", j=4, p=128)
    dn6_t = dn6_p.ap().rearrange("r (t n) -> t r n", n=NTILE)
    xT_t = xT_p.ap().rearrange("(k p) b -> k p b", p=128)  # [4, 128, B]
    xrow_t = xrow_p.ap().rearrange("(i p) d -> i p d", p=128)
    out_t = out.ap().rearrange("(i p) d -> i p d", p=128)

    psl_bufs = 2 if PIPE_V3 else BUFS_PSL
    tiny_bufs = 24 if (PIPE_V3 and V3_SPLITB) else BUFS_TINY
    dt_bufs = 4 if PIPE_V3 else BUFS_DT
    nat_bufs = 5 if PIPE_V3 else BUFS_NAT
    with tile.TileContext(nc, linearize=LINEARIZE) as tc:
        with (
            tc.tile_pool(name="persist", bufs=1) as pp,
            tc.tile_pool(name="xf", bufs=2) as xfp,
            tc.tile_pool(name="dtf", bufs=dt_bufs) as dtfp,
            tc.tile_pool(name="dtr", bufs=dt_bufs) as dtrp,
            tc.tile_pool(name="nat", bufs=nat_bufs) as natp,
            tc.tile_pool(name="dn", bufs=nat_bufs) as dnp,
            tc.tile_pool(name="sf", bufs=BUFS_SF) as sfp,
            tc.tile_pool(name="tiny", bufs=tiny_bufs) as tp,
            tc.tile_pool(name="fin", bufs=2) as finp,
            tc.tile_pool(name="psL", bufs=psl_bufs, space="PSUM") as psL,
            tc.tile_pool(name="psA", bufs=BUFS_PSA, space="PSUM") as psA,
            tc.tile_pool(name="psP", bufs=BUFS_PSP, space="PSUM") as psP,
            tc.tile_pool(name="dram", bufs=1, space="DRAM") as dram,
        ):
            # ---------------- constants / setup ----------------
            ident = pp.tile([128, 128], F32)
            make_identity(nc, ident[:])
            ident_bf = pp.tile([128, 128], BF16)
            nc.vector.tensor_copy(ident_bf[:], ident[:])

            ucol = pp.tile([128, NBT], F32)
            nucol = pp.tile([128, NBT], F32)
            dcol = pp.tile([128, NBT], F32)
            vcol = pp.tile([128, NBT], F32)
            for t_, p_ in ((ucol, ucol_p), (nucol, nucol_p), (dcol, dcol_p), (vcol, vcol_p)):
                nc.sync.dma_start(out=t_[:], in_=p_.ap())

            w6 = pp.tile([6, B], BF16)
            nc.sync.dma_start(out=w6[:], in_=w6_p.ap())

            # xT chunks, rounded to f32r once
            MMDT = BF16 if STRIP == "mm1bf" else (F16 if FP16_DT else F32R)
            xr = [pp.tile([128, B], MMDT, tag=f"xr{k}", name=f"xr{k}") for k in range(4)]
            for k in range(4):
                xf = xfp.tile([128, B], F32, tag="xf")
                nc.sync.dma_start(out=xf[:], in_=xT_t[k])
                nc.vector.tensor_copy(xr[k][:], xf[:])

            # running stats
            m_run = pp.tile([128, NBT], F32)
            l_run = pp.tile([128, NBT], F32)
            acc = [pp.tile([128, D], F32, tag=f"acc{i}", name=f"acc{i}") for i in range(NBT)]

            def emit_reset():
                nc.vector.memset(m_run[:], -1.0e30)
                nc.vector.memset(l_run[:], 0.0)
                for i in range(NBT):
                    nc.vector.memset(acc[i][:], 0.0)

            # ---------------- main loop over dataset tiles ----------------
            # Software-pipelined: MM1 of b-tile i is emitted before the
            # softmax tail of b-tile i-1, so PE overlaps the DVE/ACT chain.
            # (emit_* below; emit_body() emits the whole computation once)

            def emit_mm1(i, dTr_all, dn6t):
                if STRIP == "load":
                    return None
                bi = slice(i * 128, (i + 1) * 128)
                pL = psL.tile([128, NTILE], F32, tag="pL")
                for k in range(4):
                    nc.tensor.matmul(
                        pL[:], xr[k][:, bi], dTr_all[:, k * NTILE:(k + 1) * NTILE],
                        start=(k == 0), stop=False,
                    )
                nc.tensor.matmul(pL[:], w6[:, bi], dn6t[:], start=False, stop=True)
                return pL

            def emit_stats(i, pL):
                if STRIP in ("mm1", "mm1bf") or pL is None:
                    return None, None
                if STRIP == "exponly":  # timing probe: exp straight off MM1
                    prob = sfp.tile([128, NTILE], BF16, tag="prob")
                    lt = tp.tile([128, 1], F32, tag="lt")
                    nc.scalar.activation(
                        prob[:], pL[:], AF.Exp, bias=0.0, scale=-0.001,
                        accum_out=lt[:],
                    )
                    return None, None
                # online max update
                mt = tp.tile([128, 1], F32, tag="mt")
                nc.vector.tensor_reduce(mt[:], pL[:], axis=AX.X, op=OP.max)
                if STRIP == "red":
                    return None, None
                if STRIP == "redex":  # timing probe: reduce + exp, no smalls
                    prob = sfp.tile([128, NTILE], BF16, tag="prob")
                    lt = tp.tile([128, 1], F32, tag="lt")
                    nc.scalar.activation(
                        prob[:], pL[:], AF.Exp, bias=0.0, scale=-0.001,
                        accum_out=lt[:],
                    )
                    return None, None
                dlt = tp.tile([128, 1], F32, tag="dlt")
                # dlt = min(m_old - mt, 0) = m_old - m_new
                nc.vector.tensor_scalar(
                    out=dlt[:], in0=m_run[:, i:i + 1], scalar1=mt[:],
                    scalar2=0.0, op0=OP.subtract, op1=OP.min,
                )
                if SMALL_ON_POOL:
                    nc.gpsimd.tensor_tensor(
                        m_run[:, i:i + 1], m_run[:, i:i + 1], mt[:], op=OP.max
                    )
                else:
                    nc.vector.tensor_tensor(
                        m_run[:, i:i + 1], m_run[:, i:i + 1], mt[:], op=OP.max
                    )
                alpha = tp.tile([128, 1], F32, tag="alpha")
                nc.scalar.activation(
                    alpha[:], dlt[:], AF.Exp, bias=0.0, scale=ucol[:, i:i + 1]
                )
                # bias = -u * m_new
                ebias = tp.tile([128, 1], F32, tag="ebias")
                nc.vector.tensor_tensor(
                    ebias[:], nucol[:, i:i + 1], m_run[:, i:i + 1], op=OP.mult
                )
                if STRIP == "redsm":
                    return None, None
                # prob = exp(u*P + bias), lt = rowsum
                prob = sfp.tile([128, NTILE], BF16, tag="prob")
                lt = tp.tile([128, 1], F32, tag="lt")
                if STRIP == "statsna":  # timing probe: exp without accum_out
                    nc.scalar.activation(
                        prob[:], pL[:], AF.Exp,
                        bias=ebias[:], scale=ucol[:, i:i + 1],
                    )
                    nc.vector.tensor_reduce(lt[:], prob[:], axis=AX.X, op=OP.add)
                elif STRIP == "statset":  # timing probe: plain exp, const scale/bias
                    nc.scalar.activation(prob[:], pL[:], AF.Exp, bias=0.0, scale=-0.001)
                    nc.vector.tensor_reduce(lt[:], prob[:], axis=AX.X, op=OP.add)
                else:
                    nc.scalar.activation(
                        prob[:], pL[:], AF.Exp,
                        bias=ebias[:], scale=ucol[:, i:i + 1], accum_out=lt[:],
                    )
                # l = l*alpha + lt (fused)
                nc.vector.scalar_tensor_tensor(
                    out=l_run[:, i:i + 1], in0=l_run[:, i:i + 1],
                    scalar=alpha[:], in1=lt[:], op0=OP.mult, op1=OP.add,
                )
                return prob, alpha

            def emit_transpose(i, prob):
                if STRIP in ("stats", "statsna", "statset") or prob is None:
                    return None
                # probT transpose (bf16): xbar DMA or PE + copy
                probT = sfp.tile([128, NTILE], BF16, tag="probT")
                if USE_XBAR:
                    for k in range(0, NTILE, 128):
                        ksl = slice(k, k + 128)
                        nc.sync.dma_start_transpose(probT[:, ksl], prob[:, ksl])
                else:
                    pP = psP.tile([128, NTILE], BF16, tag="pP", name="pP")
                    for k in range(0, NTILE, 128):
                        ksl = slice(k, k + 128)
                        nc.tensor.transpose(pP[:, ksl], prob[:, ksl], ident_bf[:])
                    _copy(nc, PROBT_ENGINE, probT[:], pP[:])
                return probT

            def emit_mm2(i, probT, alpha, natbf_all):
                if STRIP != "full" or probT is None:
                    return
                # MM2: pA = probT-chunks @ natbf; acc = alpha*acc + pA (DVE)
                pA = psA.tile([128, D], F32, tag="pA")
                for k in range(4):
                    ksl = slice(k * 128, (k + 1) * 128)
                    nc.tensor.matmul(
                        pA[:], probT[:, ksl],
                        natbf_all[:, k * D:(k + 1) * D],
                        start=(k == 0), stop=(k == 3),
                    )
                nc.vector.scalar_tensor_tensor(
                    out=acc[i][:], in0=acc[i][:],
                    scalar=alpha[:], in1=pA[:], op0=OP.mult, op1=OP.add,
                )

            def emit_loads(t):
                natbf_all = natp.tile([128, 4 * D], BF16, tag="natbf")
                dTr_all = dtrp.tile([128, 4 * NTILE], MMDT, tag="dTr")
                nc.sync.dma_start(
                    out=natbf_all[:].rearrange("p (j d) -> p j d", j=4),
                    in_=natb_t[t],
                )
                if FP16_DT:
                    nc.sync.dma_start(
                        out=dTr_all[:].rearrange("p (k n) -> p k n", k=4),
                        in_=dT_t[t],
                    )
                else:
                    dTf_all = dtfp.tile([128, 4 * NTILE], F32, tag="dTf")
                    nc.sync.dma_start(
                        out=dTf_all[:].rearrange("p (k n) -> p k n", k=4),
                        in_=dT_t[t],
                    )
                    _copy(nc, ROUND_ENGINE, dTr_all[:], dTf_all[:])
                dn6t = dnp.tile([6, NTILE], BF16, tag="dn6t")
                nc.sync.dma_start(out=dn6t[:], in_=dn6_t[t])
                return natbf_all, dTr_all, dn6t

            # ---------------- PIPE_V2: stage-split pipeline ----------------
            # Key fix over v1: the per-block stats chain used to round-trip
            # DVE -> ACT -> DVE (reduce/smalls -> alpha+exp -> l_run) inside
            # one emission slot, so each engine's in-order queue stalled on
            # the other engine every block.  v2 splits consumers of fresh
            # results into later slots: alpha/l_run (statsB) and the acc
            # update run several blocks later, when their inputs are long
            # since ready.  Emission order per step s:
            #   MM1(s) | acc(s-5) | MM2(s-4) | statsB(s-3) | trans(s-2)
            #   | statsA(s-1)
            def emit_statsA(i, pL):
                """reduce -> dlt -> m_run max -> ebias -> exp (prob, lt)."""
                if STRIP in ("mm1", "mm1bf") or pL is None:
                    return None
                mt = tp.tile([128, 1], F32, tag="mt")
                nc.vector.tensor_reduce(mt[:], pL[:], axis=AX.X, op=OP.max)
                if STRIP == "red":
                    return None
                dlt = tp.tile([128, 1], F32, tag="dlt")
                nc.vector.tensor_scalar(
                    out=dlt[:], in0=m_run[:, i:i + 1], scalar1=mt[:],
                    scalar2=0.0, op0=OP.subtract, op1=OP.min,
                )
                nc.vector.tensor_tensor(
                    m_run[:, i:i + 1], m_run[:, i:i + 1], mt[:], op=OP.max
                )
                ebias = tp.tile([128, 1], F32, tag="ebias")
                nc.vector.tensor_tensor(
                    ebias[:], nucol[:, i:i + 1], m_run[:, i:i + 1], op=OP.mult
                )
                if STRIP == "redsm":
                    return None
                prob = sfp.tile([128, NTILE], BF16, tag="prob")
                lt = tp.tile([128, 1], F32, tag="lt")
                nc.scalar.activation(
                    prob[:], pL[:], AF.Exp,
                    bias=ebias[:], scale=ucol[:, i:i + 1], accum_out=lt[:],
                )
                return dict(prob=prob, dlt=dlt, lt=lt)

            def emit_statsB(i, st):
                """alpha (ACT) + l_run update (DVE) — inputs are 2 slots old."""
                if st is None:
                    return None
                alpha = tp.tile([128, 1], F32, tag="alpha")
                nc.scalar.activation(
                    alpha[:], st["dlt"][:], AF.Exp, bias=0.0,
                    scale=ucol[:, i:i + 1]
                )
                nc.vector.scalar_tensor_tensor(
                    out=l_run[:, i:i + 1], in0=l_run[:, i:i + 1],
                    scalar=alpha[:], in1=st["lt"][:], op0=OP.mult, op1=OP.add,
                )
                return alpha

            def emit_mm2_v2(i, probT, natbf_all):
                if STRIP != "full" or probT is None:
                    return None
                pA = psA.tile([128, D], F32, tag="pA")
                for k in range(4):
                    ksl = slice(k * 128, (k + 1) * 128)
                    nc.tensor.matmul(
                        pA[:], probT[:, ksl],
                        natbf_all[:, k * D:(k + 1) * D],
                        start=(k == 0), stop=(k == 3),
                    )
                return pA

            def emit_acc(i, pA, alpha):
                if pA is None or alpha is None:
                    return
                nc.vector.scalar_tensor_tensor(
                    out=acc[i][:], in0=acc[i][:],
                    scalar=alpha[:], in1=pA[:], op0=OP.mult, op1=OP.add,
                )

            def emit_body_v2():
                emit_reset()
                blk = []

                def stage(s):
                    if 0 <= s - 5 < len(blk):
                        b = blk[s - 5]
                        emit_acc(b["i"], b.get("pA"), b.get("alpha"))
                    if 0 <= s - 4 < len(blk):
                        b = blk[s - 4]
                        b["pA"] = emit_mm2_v2(b["i"], b.get("probT"), b["nat"])
                    if 0 <= s - 3 < len(blk):
                        b = blk[s - 3]
                        b["alpha"] = emit_statsB(b["i"], b.get("st"))
                    if 0 <= s - 2 < len(blk):
                        b = blk[s - 2]
                        st = b.get("st")
                        b["probT"] = emit_transpose(
                            b["i"], st["prob"] if st else None
                        )
                    if 0 <= s - 1 < len(blk):
                        b = blk[s - 1]
                        b["st"] = emit_statsA(b["i"], b["pL"])

                nxt = emit_loads(0)
                s = 0
                for t in range(n_tiles):
                    cur = nxt
                    for i in range(NBT):
                        pL = emit_mm1(i, cur[1], cur[2])
                        blk.append(dict(i=i, pL=pL, nat=cur[0]))
                        stage(s)
                        s += 1
                        if i == 1 and t + 1 < n_tiles:
                            nxt = emit_loads(t + 1)
                for s2 in range(s, s + 6):
                    stage(s2)

            def emit_body():
                emit_reset()
                # per-engine staged pipeline (HW engines are in-order; stages
                # must lag enough that their inputs are long since ready):
                #   iter j: MM1(j), stats(j-STATS_LAG),
                #   transposes(j-STATS_LAG-TRANS_LAG), MM2(two lags further)
                nxt = emit_loads(0)
                stats_q = []  # (i, pL, natbf_all)
                trans_q = []  # (i, prob, alpha, natbf_all)
                mm2_q = []  # (i, probT, alpha, natbf_all)

                def step_queues(drain=False):
                    if len(mm2_q) >= (1 if drain else MM2_LAG):
                        emit_mm2(*mm2_q.pop(0))
                    if not FUSE_TRANS and len(trans_q) >= (1 if drain else TRANS_LAG):
                        ci, cprob, calpha, cnat = trans_q.pop(0)
                        probT = emit_transpose(ci, cprob)
                        mm2_q.append((ci, probT, calpha, cnat))
                    if len(stats_q) >= (1 if drain else STATS_LAG):
                        ci, cpL, cnat = stats_q.pop(0)
                        prob, alpha = emit_stats(ci, cpL)
                        if FUSE_TRANS:
                            probT = emit_transpose(ci, prob)
                            mm2_q.append((ci, probT, alpha, cnat))
                        else:
                            trans_q.append((ci, prob, alpha, cnat))

                for t in range(n_tiles):
                    cur = nxt
                    for i in range(NBT):
                        pL = emit_mm1(i, cur[1], cur[2])
                        step_queues()
                        stats_q.append((i, pL, cur[0]))
                        if i == 1 and t + 1 < n_tiles:
                            nxt = emit_loads(t + 1)
                while stats_q or trans_q or mm2_q:
                    step_queues(drain=True)


            # ---------------- PIPE_V3: paired n-tiles ----------------
            # One [128, 2*NTILE] pL spanning 2 PSUM banks per (b-tile, tile
            # pair): halves the count of reduce/exp/smalls/acc instructions
            # and amortizes each weight load over two streams (A/B emitted
            # back-to-back under the same lhsT).
            def emit_mm1_v3(i, la, lb):
                # la/lb = (natbf, dTr, dn6) for tile A / B (lb None on tail)
                if STRIP == "load":
                    return None
                nb = 2 if lb is not None else 1
                bi = slice(i * 128, (i + 1) * 128)
                pL = psL.tile([128, 2 * NTILE], F32, tag="pL2")
                regs = [pL[:, :NTILE], pL[:, NTILE:]][:nb]
                dts = [la[1], lb[1] if lb else None]
                for k in range(4):
                    for r in range(nb):
                        nc.tensor.matmul(
                            regs[r], xr[k][:, bi],
                            dts[r][:, k * NTILE:(k + 1) * NTILE],
                            start=(k == 0), stop=False,
                        )
                for r in range(nb):
                    nc.tensor.matmul(
                        regs[r], w6[:, bi], (la if r == 0 else lb)[2][:],
                        start=False, stop=True,
                    )
                return pL

            def emit_statsA_v3(i, pL, nb):
                if STRIP in ("mm1", "mm1bf") or pL is None:
                    return None
                w = nb * NTILE
                mt = tp.tile([128, 1], F32, tag="mt")
                nc.vector.tensor_reduce(mt[:], pL[:, :w], axis=AX.X, op=OP.max)
                if STRIP == "red":
                    return None
                dlt = tp.tile([128, 1], F32, tag="dlt")
                nc.vector.tensor_scalar(
                    out=dlt[:], in0=m_run[:, i:i + 1], scalar1=mt[:],
                    scalar2=0.0, op0=OP.subtract, op1=OP.min,
                )
                nc.vector.tensor_tensor(
                    m_run[:, i:i + 1], m_run[:, i:i + 1], mt[:], op=OP.max
                )
                alpha = None
                if not V3_SPLITB:
                    alpha = tp.tile([128, 1], F32, tag="alpha")
                    nc.scalar.activation(
                        alpha[:], dlt[:], AF.Exp, bias=0.0,
                        scale=ucol[:, i:i + 1]
                    )
                ebias = tp.tile([128, 1], F32, tag="ebias")
                nc.vector.tensor_tensor(
                    ebias[:], nucol[:, i:i + 1], m_run[:, i:i + 1], op=OP.mult
                )
                if STRIP == "redsm":
                    return None
                prob = sfp.tile([128, 2 * NTILE], BF16, tag="prob2")
                lt = tp.tile([128, 1], F32, tag="lt")
                nc.scalar.activation(
                    prob[:, :w], pL[:, :w], AF.Exp,
                    bias=ebias[:], scale=ucol[:, i:i + 1], accum_out=lt[:],
                )
                if not V3_SPLITB:
                    nc.vector.scalar_tensor_tensor(
                        out=l_run[:, i:i + 1], in0=l_run[:, i:i + 1],
                        scalar=alpha[:], in1=lt[:], op0=OP.mult, op1=OP.add,
                    )
                return prob, alpha, dlt, lt

            def emit_transpose_v3(i, prob, nb):
                if STRIP in ("stats", "statsna", "statset") or prob is None:
                    return None
                w = nb * NTILE
                probT = sfp.tile([128, 2 * NTILE], BF16, tag="probT2")
                pP = psP.tile([128, 2 * NTILE], BF16, tag="pP2", name="pP2")
                for k in range(0, w, 128):
                    ksl = slice(k, k + 128)
                    nc.tensor.transpose(pP[:, ksl], prob[:, ksl], ident_bf[:])
                if V3_COPYSPLIT:
                    half = w // 2
                    nc.scalar.copy(probT[:, :half], pP[:, :half])
                    nc.vector.tensor_copy(probT[:, half:w], pP[:, half:w])
                else:
                    _copy(nc, PROBT_ENGINE, probT[:, :w], pP[:, :w])
                return probT

            def emit_mm2_v3(i, probT, alpha, dlt, lt, la, lb, nb):
                if STRIP != "full" or probT is None:
                    return None
                if V3_SPLITB:
                    alpha = tp.tile([128, 1], F32, tag="alpha")
                    nc.scalar.activation(
                        alpha[:], dlt[:], AF.Exp, bias=0.0,
                        scale=ucol[:, i:i + 1]
                    )
                    nc.vector.scalar_tensor_tensor(
                        out=l_run[:, i:i + 1], in0=l_run[:, i:i + 1],
                        scalar=alpha[:], in1=lt[:], op0=OP.mult, op1=OP.add,
                    )
                pA = psA.tile([128, D], F32, tag="pA")
                nk = 4 * nb
                for k in range(nk):
                    ksl = slice(k * 128, (k + 1) * 128)
                    nat = la[0] if k < 4 else lb[0]
                    nc.tensor.matmul(
                        pA[:], probT[:, ksl],
                        nat[:, (k % 4) * D:((k % 4) + 1) * D],
                        start=(k == 0), stop=(k == nk - 1),
                    )
                if not V3_LATEACC:
                    nc.vector.scalar_tensor_tensor(
                        out=acc[i][:], in0=acc[i][:],
                        scalar=alpha[:], in1=pA[:], op0=OP.mult, op1=OP.add,
                    )
                    return None
                return (i, pA, alpha)

            def emit_acc_v3(i, pA, alpha):
                nc.vector.scalar_tensor_tensor(
                    out=acc[i][:], in0=acc[i][:],
                    scalar=alpha[:], in1=pA[:], op0=OP.mult, op1=OP.add,
                )

            def emit_body_v3():
                emit_reset()
                pairs = [(t, t + 1 if t + 1 < n_tiles else None)
                         for t in range(0, n_tiles, 2)]
                loads = {0: emit_loads(0)}
                if n_tiles > 1:
                    loads[1] = emit_loads(1)
                stats_q = []  # (i, pL, nb, la, lb)
                mm2_q = []  # (i, probT, alpha, la, lb, nb)
                acc_q = []  # (i, pA, alpha) pending acc STTs

                def step_queues(drain=False):
                    if acc_q:
                        emit_acc_v3(*acc_q.pop(0))
                    if len(mm2_q) >= (1 if drain else V3_MM2_LAG):
                        r = emit_mm2_v3(*mm2_q.pop(0))
                        if r is not None:
                            acc_q.append(r)
                    if len(stats_q) >= (1 if drain else V3_STATS_LAG):
                        ci, cpL, cnb, cla, clb = stats_q.pop(0)
                        res = emit_statsA_v3(ci, cpL, cnb)
                        prob, alpha, dlt, lt = res if res else (None,) * 4
                        probT = emit_transpose_v3(ci, prob, cnb)
                        mm2_q.append(
                            (ci, probT, alpha, dlt, lt, cla, clb, cnb)
                        )

                for p, (ta, tb) in enumerate(pairs):
                    la = loads.pop(ta)
                    lb = loads.pop(tb, None) if tb is not None else None
                    nb = 2 if lb is not None else 1
                    for i in range(NBT):
                        pL = emit_mm1_v3(i, la, lb)
                        step_queues()
                        stats_q.append((i, pL, nb, la, lb))
                        nxt_t = 2 * (p + 1) + (0 if i == 1 else 1)
                        if i in (1, 4) and nxt_t < n_tiles and nxt_t not in loads:
                            loads[nxt_t] = emit_loads(nxt_t)
                while stats_q or mm2_q or acc_q:
                    step_queues(drain=True)

            def emit_merge():
                    # ---------------- cross-core merge ----------------
                    m_cc_in = dram.tile([128, NBT], F32)
                    m_cc_out = dram.tile([128, NBT], F32)
                    nc.sync.dma_start(out=m_cc_in[:], in_=m_run[:])
                    if not SIM_1CORE:
                        nc.gpsimd.collective_compute(
                            "AllReduce", OP.max,
                            replica_groups=[list(range(NCORES))],
                            ins=[m_cc_in[:].opt()], outs=[m_cc_out[:].opt()],
                        )
                    else:
                        nc.sync.dma_start(out=m_cc_out[:], in_=m_cc_in[:])
                    m_glob = pp.tile([128, NBT], F32)
                    nc.sync.dma_start(out=m_glob[:], in_=m_cc_out[:])

                    # gamma_i = exp(u * (m_loc - m_glob)); scale acc, l
                    dg = pp.tile([128, NBT], F32)
                    nc.vector.tensor_tensor(dg[:], m_run[:], m_glob[:], op=OP.subtract)
                    gam = pp.tile([128, NBT], F32)
                    for i in range(NBT):
                        nc.scalar.activation(
                            gam[:, i:i + 1], dg[:, i:i + 1], AF.Exp,
                            bias=0.0, scale=ucol[:, i:i + 1],
                        )
                    nc.vector.tensor_tensor(l_run[:], l_run[:], gam[:], op=OP.mult)

                    accl_in = dram.tile([128, NBT * D + NBT], BF16)
                    accl_out = dram.tile([128, NBT * D + NBT], BF16)
                    for i in range(NBT):
                        accs = finp.tile([128, D], BF16, tag="accs")
                        nc.vector.tensor_scalar(
                            out=accs[:], in0=acc[i][:], scalar1=gam[:, i:i + 1],
                            scalar2=None, op0=OP.mult,
                        )
                        nc.sync.dma_start(out=accl_in[:, i * D:(i + 1) * D], in_=accs[:])
                    lbf = finp.tile([128, NBT], BF16, tag="lbf")
                    nc.vector.tensor_copy(lbf[:], l_run[:])
                    nc.sync.dma_start(out=accl_in[:, NBT * D:], in_=lbf[:])
                    if not SIM_1CORE:
                        nc.gpsimd.collective_compute(
                            "AllReduce", OP.add,
                            replica_groups=[list(range(NCORES))],
                            ins=[accl_in[:].opt()], outs=[accl_out[:].opt()],
                        )
                    else:
                        nc.sync.dma_start(out=accl_out[:], in_=accl_in[:])

                    lg = pp.tile([128, NBT], BF16)
                    nc.sync.dma_start(out=lg[:], in_=accl_out[:, NBT * D:])
                    rl = pp.tile([128, NBT], F32)
                    nc.vector.reciprocal(rl[:], lg[:])
                    # s1 = dcoef / l
                    s1 = pp.tile([128, NBT], F32)
                    nc.vector.tensor_tensor(s1[:], dcol[:], rl[:], op=OP.mult)
                    for i in range(NBT):
                        accg = finp.tile([128, D], BF16, tag="accg")
                        nc.sync.dma_start(out=accg[:], in_=accl_out[:, i * D:(i + 1) * D])
                        xnat = xfp.tile([128, D], F32, tag="xnat")
                        nc.sync.dma_start(out=xnat[:], in_=xrow_t[i])
                        v1 = finp.tile([128, D], F32, tag="v1")
                        nc.vector.tensor_scalar(
                            out=v1[:], in0=accg[:], scalar1=s1[:, i:i + 1],
                            scalar2=None, op0=OP.mult,
                        )
                        v2 = finp.tile([128, D], F32, tag="v2")
                        nc.vector.tensor_scalar(
                            out=v2[:], in0=xnat[:], scalar1=vcol[:, i:i + 1],
                            scalar2=None, op0=OP.mult,
                        )
                        nc.vector.tensor_tensor(v1[:], v1[:], v2[:], op=OP.add)
                        nc.sync.dma_start(out=out_t[i], in_=v1[:])

            body = emit_body_v3 if PIPE_V3 else (
                emit_body_v2 if PIPE_V2 else emit_body
            )
            if repeat == 1:
                body()
            else:
                with tc.For_i(0, repeat):
                    body()
            for _ in range(merge_repeat):
                emit_merge()

    nc.compile()
    return nc


_BUILD_CACHE = {}


def _get_nc(n_tiles, repeat=1, merge_repeat=1):
    key = (n_tiles, repeat, merge_repeat, STRIP, PIPE_V2, PIPE_V3, V3_STATS_LAG, V3_MM2_LAG, V3_SPLITB, FP16_DT, V3_LATEACC, V3_COPYSPLIT, STATS_LAG, TRANS_LAG, MM2_LAG, FUSE_TRANS, SMALL_ON_POOL, SIM_1CORE, LINEARIZE, ROUND_ENGINE, PROBT_ENGINE, ACC_ENGINE, USE_XBAR,
           BUFS_DT, BUFS_NAT, BUFS_SF, BUFS_TINY, BUFS_PSL, BUFS_PSA, BUFS_PSP)
    if key not in _BUILD_CACHE:
        _BUILD_CACHE[key] = build(n_tiles, repeat, merge_repeat)
    return _BUILD_CACHE[key]


def _bf16(x):
    """Fast round-to-nearest-even fp32 -> bf16 (returns ml_dtypes.bfloat16)."""
    u = np.ascontiguousarray(x, dtype=np.float32).view(np.uint32)
    r = (u + np.uint32(0x7FFF) + ((u >> np.uint32(16)) & np.uint32(1))) >> np.uint32(16)
    return r.astype(np.uint16).view(ml_dtypes.bfloat16)


def _split3_bf16(v):
    """3-way bf16 split of a float64/float32 vector: v ~ s1+s2+s3."""
    v = v.astype(np.float32)
    s1 = _bf16(v)
    r1 = v - s1.astype(np.float32)
    s2 = _bf16(r1)
    r2 = r1 - s2.astype(np.float32)
    s3 = _bf16(r2)
    return s1, s2, s3


def make_in_maps(x_t, t, dataset, n_tiles):
    """Shard + pad dataset, precompute transposes/casts/norms + coeffs."""
    n = dataset.shape[0]
    n_sh = n_tiles * NTILE
    n_pad = NCORES * n_sh
    assert n_pad >= n
    dpad = np.zeros((n_pad, D), dtype=np.float32)
    dpad[:n] = dataset
    dpad[n:, 0] = 1000.0  # far-away pad rows: huge norm, ~zero softmax weight

    a = t.astype(np.float64)
    b = 1.0 - a
    u = (a / (b * b)).astype(np.float32)
    w = -a / 2.0
    dcoef = (1.0 + a / b).astype(np.float32)
    vcoef = (-1.0 / b).astype(np.float32)

    w1, w2, w3 = _split3_bf16(w)
    w6 = np.stack([w1, w1, w1, w2, w2, w3])  # [6, B] bf16

    # f32 inputs with f64 accumulation: bit-identical to the all-f64 einsum
    # (f32*f32 products are exact in f64) without materializing 400MB casts
    dn = np.einsum("nd,nd->n", dpad, dpad, dtype=np.float64)
    d1, d2, d3 = _split3_bf16(dn)
    dn6 = np.stack([d1, d2, d3, d1, d2, d1])  # [6, n_pad] bf16

    dT = np.ascontiguousarray(dpad.T)  # [D, n_pad]
    if FP16_DT:
        dT = dT.astype(np.float16)
    natb = _bf16(dpad).reshape(n_pad, D)

    def col(v):
        return np.ascontiguousarray(v.reshape(NBT, 128).T)

    base = dict(
        xT=np.ascontiguousarray(x_t.T),
        xrow=np.ascontiguousarray(x_t),
        w6=np.ascontiguousarray(w6),
        ucol=col(u),
        nucol=col(-u),
        dcol=col(dcoef),
        vcol=col(vcoef),
    )
    return [
        dict(
            base,
            dT=np.ascontiguousarray(dT[:, c * n_sh:(c + 1) * n_sh]),
            natb=np.ascontiguousarray(natb[c * n_sh:(c + 1) * n_sh]),
            dn6=np.ascontiguousarray(dn6[:, c * n_sh:(c + 1) * n_sh]),
        )
        for c in range(NCORES)
    ]


def kernel(x_t, t, dataset):
    x_t = np.asarray(x_t, dtype=np.float32)
    t = np.asarray(t, dtype=np.float32)
    dataset = np.asarray(dataset, dtype=np.float32)
    n = dataset.shape[0]
    n_tiles = -(-n // (NCORES * NTILE))  # ceil -> 25 for N=100000
    nc = _get_nc(n_tiles)
    in_maps = make_in_maps(x_t, t, dataset, n_tiles)
    res = run_bass_kernel_spmd(nc, in_maps, core_ids=list(range(NCORES)))
    return np.asarray(res.results[0]["out"], dtype=np.float32)


def ref_numpy(x_t, t, dataset):
    aa = t.astype(np.float64)
    bb = 1.0 - aa
    dsn = (dataset.astype(np.float64) ** 2).sum(1)
    t2 = x_t.astype(np.float64) @ dataset.T.astype(np.float64)
    logit = (-1.0 / (2 * bb * bb))[:, None] * (
        (x_t.astype(np.float64) ** 2).sum(1)[:, None]
        - 2 * aa[:, None] * t2
        + (aa * aa)[:, None] * dsn[None, :]
    )
    p = np.exp(logit - logit.max(1, keepdims=True))
    p /= p.sum(1, keepdims=True)
    p = p * (1 + aa / bb)[:, None]
    return (-1.0 / bb)[:, None] * x_t.astype(np.float64) + p @ dataset.astype(np.float64)


if __name__ == "__main__":
    rng = np.random.default_rng(0)
    n = 2 * NCORES * NTILE - 300
    x_t = rng.standard_normal((B, D)).astype(np.float32)
    t = rng.uniform(0.05, 0.95, (B,)).astype(np.float32)
    dataset = rng.standard_normal((n, D)).astype(np.float32)
    v = kernel(x_t, t, dataset)
    vref = ref_numpy(x_t, t, dataset)
    err = np.linalg.norm(v - vref) / np.linalg.norm(vref)
    print("rel l2 err:", err)
    print("max abs err:", np.abs(v - vref).max(), "ref scale:", np.abs(vref).max())

